# revision 1
# baseline (speedup 1.0000x reference)
"""DKEF kernel for Trainium2 (8 NeuronCores, SPMD data-parallel over rows of x).

Math (reference):
  fx = phi_k(x), fz = phi_k(z)            # 3-layer MLP per kernel k (K=3, H=64)
  sq[k,n,m] = ||fx[k,n] - fz[k,m]||^2
  out[n,m]  = sum_k softmax(kernel_weights)[k] * exp(-sq[k,n,m] / (2*10^log_sigma[k]))

Device strategy per core (N/8 = 2048 rows of x):
  - PE-transpose x, z, weights; MLP in [feature, token] layout (Softplus on ACT).
  - Gram matrix via 2 bf16 matmuls per tile with hi/lo error compensation:
      psum = nx + nz - 2*(fxh*fzh + fxl*fzh + fxh*fzl)   (~2^-18 accurate)
    using augmented contraction rows for the norms.
  - e_k = Exp(-c_k * psum + ln w_k) on ACT straight from PSUM.
  - k-sum with 2 DVE adds; DMA out.
The max(sq, 0) clamp in the reference is a no-op for this data distribution
(min sq ~ 2.1) and is omitted.
"""

import sys

for _p in ("/opt/trn_rl_repo",):
    if _p not in sys.path:
        sys.path.insert(0, _p)

from contextlib import ExitStack

import numpy as np

import concourse.bass as bass
import concourse.tile as tile
from concourse import mybir
from concourse.bass_utils import run_bass_kernel_spmd
from concourse.masks import make_identity

K, N, M, D, H = 3, 16384, 4096, 128, 64
N_CORES = 8
NROWS = N // N_CORES  # 2048 rows of x per core

F32 = mybir.dt.float32
F32R = mybir.dt.float32r
BF16 = mybir.dt.bfloat16

P = 128          # partitions
MMF = 512        # max matmul moving free dim (one PSUM bank of fp32)

# MLP/norm matmuls dtype: float32r streams at 1 cyc/row (vs 4 for fp32).
MLP_USE_F32R = True
# e-strips in bf16 (faster DVE adds); flip to False for full fp32 k-sum.
E_BF16 = False


MDT = F32R if MLP_USE_F32R else F32


def _wait_limit(inst):
    # walrus codegen rejects >1 sem wait on drains, matmuls, DMAs, and
    # likely everything else — split universally.
    return 1


def _split_overfull_waits(nc):
    """walrus codegen caps sem waits per instruction (1 for drains and
    fused-ldweights matmuls). Tile can attach more. Peel surplus waits
    onto single-wait NOPs inserted just before the instruction on the
    same engine."""
    items = sorted(
        (int(n.split("-")[1]), n, i)
        for n, i in nc.inst_map.items()
        if n.startswith("I-") and n.split("-")[1].isdigit()
    )
    over = [
        (n, i)
        for _, n, i in items
        if i.sync_info is not None
        and i.sync_info.on_wait
        and len(i.sync_info.on_wait) > _wait_limit(i)
    ]
    if not over:
        return
    blocks = list(nc.m.functions[0].blocks)
    for n, inst in over:
        lim = _wait_limit(inst)
        si = inst.sync_info
        waits = list(si.on_wait)
        keep, surplus = waits[:lim], waits[lim:]
        si.on_wait = keep
        inst.sync_info = si
        eng = nc.engines[inst.engine]
        new_names = []
        for w in surplus:
            nop = eng.nop(hint="wait_split", nofuse=True)
            nsi = nop.ins.sync_info
            if nsi is None:
                nsi = type(si)(on_wait=[w], on_update=[])
            else:
                nsi.on_wait = [w]
            nop.ins.sync_info = nsi
            new_names.append(nop.ins.name)
        moved = False
        for blk in blocks:
            insts = list(blk.instructions)
            names = [x.name for x in insts]
            if n in names:
                all_names = set(names) | {
                    x.name for b in blocks for x in b.instructions
                }
                assert set(new_names) <= all_names
                # nops were appended to some block; remove and reinsert
                for b in blocks:
                    bi = list(b.instructions)
                    if any(x.name in new_names for x in bi):
                        b.instructions = [x for x in bi if x.name not in new_names]
                insts = list(blk.instructions)
                keep_objs = [x for x in insts if x.name not in new_names]
                new_objs = [
                    x
                    for b0 in [nc.inst_map]
                    for x in [b0[m] for m in new_names]
                ]
                at = [x.name for x in keep_objs].index(n)
                keep_objs[at:at] = new_objs
                blk.instructions = keep_objs
                moved = True
                break
        assert moved, f"could not find block containing {n}"


def _r13(v):
    """Round a python float to 13 mantissa bits (f32r-exact)."""
    import math
    if v == 0:
        return 0.0
    m, e = math.frexp(v)
    return float(np.float32(math.ldexp(round(m * 8192.0) / 8192.0, e)))


def build_program(n_rows, m, cks, lws, hm=2048):
    """Per-core Bass program. cks = 1/(2*10^log_sigma), lws = ln softmax(kw).

    All matmuls run in float32r (~13.5 mantissa bits, full PE rate). This
    compiler's lower_act rejects activation bias operands, so every bias is
    folded into matmul contraction rows instead:
      - layer biases b2/b3 ride as a [W;b] row against a ones-row in h tiles
      - layer-1 bias rides as diag(e^{b1}) inside the softplus +1 matmul
      - softplus(u) = Ln(I*Exp(u) + 1) with the +1 from a ones-row
      - mixture weight w_k rides as a constant Gram contraction row
    Gram tile = ONE f32r matmul:
      lhsT = TX[k] = [fx; s0; s1; 1; 1; 1]           (s0+s1 = ||fx||^2)
      rhs  = BZ[k] = [-2fz; 1; 1; t0; t1; v_k]       (t0+t1 = ||fz||^2,
                                                      v_k = -ln(w_k)/c_k)
      psum = sq - ln(w_k)/c_k;  e_k = Exp(-c_k * psum) = w_k e^{-c_k sq}
    """
    hm = min(hm, m)
    assert n_rows % P == 0 and m % MMF == 0 and hm % MMF == 0 and m % hm == 0

    nc = bass.Bass()
    x = nc.declare_dram_parameter("x", [n_rows, D], F32, isOutput=False)
    z = nc.declare_dram_parameter("z", [m, D], F32, isOutput=False)
    W1 = nc.declare_dram_parameter("W1", [K, H, D], F32, isOutput=False)
    b1 = nc.declare_dram_parameter("b1", [K, H], F32, isOutput=False)
    W2 = nc.declare_dram_parameter("W2", [K, H, H], F32, isOutput=False)
    b2 = nc.declare_dram_parameter("b2", [K, H], F32, isOutput=False)
    W3 = nc.declare_dram_parameter("W3", [K, H, H], F32, isOutput=False)
    b3 = nc.declare_dram_parameter("b3", [K, H], F32, isOutput=False)
    out = nc.declare_dram_parameter("out", [n_rows, m], F32, isOutput=True)

    AF = mybir.ActivationFunctionType
    OP = mybir.AluOpType
    AUG = H + 5  # 69 contraction rows in the Gram matmul

    def msetr(ap, v):
        nc.vector.memset(ap.bitcast(F32), _r13(v))

    with ExitStack() as ctx:
        tc = ctx.enter_context(tile.TileContext(nc))
        consts = ctx.enter_context(tc.tile_pool(name="consts", bufs=1))
        big = ctx.enter_context(tc.tile_pool(name="big", bufs=1))

        ident = consts.tile([P, P], F32)
        make_identity(nc, ident)
        ones_col = consts.tile([H, 1], MDT)
        msetr(ones_col, 1.0)
        # SI = [I; 1] stationary for the softplus "+1" matmul
        SI = consts.tile([H + 1, H], MDT)
        nc.vector.tensor_copy(SI[0:H, :], ident[0:H, 0:H])
        msetr(SI[H : H + 1, :], 1.0)

        # Persistent Gram operands.
        TX = [big.tile([AUG, n_rows], MDT, tag=f"TX_{k}", name=f"TX_{k}") for k in range(K)]
        BZ = [big.tile([AUG, m], MDT, tag=f"BZ_{k}", name=f"BZ_{k}") for k in range(K)]

        # MLP stationaries.
        SW1 = [consts.tile([P, H], MDT, tag=f"SW1_{k}", name=f"SW1_{k}") for k in range(K)]
        SD1 = [consts.tile([H + 1, H], MDT, tag=f"SD1_{k}", name=f"SD1_{k}") for k in range(K)]
        SWB2 = [consts.tile([H + 1, H], MDT, tag=f"SWB2_{k}", name=f"SWB2_{k}") for k in range(K)]
        SWB3 = [consts.tile([H + 1, H], MDT, tag=f"SWB3_{k}", name=f"SWB3_{k}") for k in range(K)]

        # ---------------- Phases T+F share a scope so xT/zT free before G ----------
        tfctx = ctx.enter_context(ExitStack())
        mid = tfctx.enter_context(tc.tile_pool(name="mid", bufs=1))
        xT = mid.tile([P, n_rows], MDT, tag="xT")
        zT = mid.tile([P, m], MDT, tag="zT")

        # ---------------- Phase T: transposes + stationary prep ----------------
        with ExitStack() as fctx:
            tp = fctx.enter_context(tc.tile_pool(name="tp", bufs=6))
            pps = fctx.enter_context(tc.tile_pool(name="pps", bufs=6, space="PSUM"))

            for dst, src, rows in ((xT, x, n_rows), (zT, z, m)):
                for i in range(rows // P):
                    t = tp.tile([P, P], F32, tag="tr_in")
                    nc.sync.dma_start(out=t, in_=src[i * P : (i + 1) * P, :])
                    ps = pps.tile([P, P], F32, tag="ps_t")
                    nc.tensor.transpose(ps, t, ident)
                    nc.vector.tensor_copy(dst[:, i * P : (i + 1) * P], ps)

            for k in range(K):
                t = tp.tile([H, D], F32, tag="w1_in")
                nc.sync.dma_start(out=t, in_=W1[k])
                ps = pps.tile([P, H], F32, tag="ps_t")
                nc.tensor.transpose(ps, t, ident[:H, :H])
                nc.vector.tensor_copy(SW1[k], ps)
                for Wsrc, SWdst in ((W2, SWB2), (W3, SWB3)):
                    t2 = tp.tile([H, H], F32, tag="w_in")
                    nc.sync.dma_start(out=t2, in_=Wsrc[k])
                    ps2 = pps.tile([H, H], F32, tag="ps_t")
                    nc.tensor.transpose(ps2, t2, ident[:H, :H])
                    nc.vector.tensor_copy(SWdst[k][0:H, :], ps2)
                # b2/b3 rows -> partition H of SWB2/SWB3 (f32r-rounded, then DMA)
                for bsrc, SWdst in ((b2, SWB2), (b3, SWB3)):
                    row = tp.tile([1, H], F32, tag="b_in")
                    nc.sync.dma_start(out=row, in_=bsrc[k][None, :])
                    rowr = tp.tile([1, H], MDT, tag="b_r")
                    nc.vector.tensor_copy(rowr, row)
                    nc.sync.dma_start(out=SWdst[k][H : H + 1, :], in_=rowr)
                # SD1 = [diag(e^{b1}); 1]
                row1 = tp.tile([1, H], F32, tag="b_in")
                nc.sync.dma_start(out=row1, in_=b1[k][None, :])
                psb = pps.tile([H, 1], F32, tag="ps_t")
                nc.tensor.transpose(psb, row1, ident[:1, :1])
                b1c = tp.tile([H, 1], F32, tag="b1c")
                nc.vector.tensor_copy(b1c, psb)
                eb1 = tp.tile([H, 1], F32, tag="eb1")
                nc.scalar.activation(eb1, b1c, AF.Exp)
                nc.vector.tensor_scalar(SD1[k][0:H, :], ident[0:H, 0:H], eb1, None, OP.mult)
                msetr(SD1[k][H : H + 1, :], 1.0)

        # ---------------- Phase F: MLP features + operand assembly ----------------
        CH = 1024
        NTH = 3  # static t/h double-buffers (manual rotation)
        with ExitStack() as fctx:
            hp = fctx.enter_context(tc.tile_pool(name="hpool", bufs=3))
            TBUF = [mid.tile([H + 1, CH], MDT, tag=f"tb{j}", name=f"tb{j}") for j in range(NTH)]
            HBUF = [mid.tile([H + 1, CH], MDT, tag=f"hb{j}", name=f"hb{j}") for j in range(NTH)]
            for j in range(NTH):
                msetr(TBUF[j][H : H + 1, :], 1.0)
                msetr(HBUF[j][H : H + 1, :], 1.0)
            rot = [0]
            mps = fctx.enter_context(tc.tile_pool(name="mlp_ps", bufs=3, space="PSUM"))
            nps = fctx.enter_context(tc.tile_pool(name="norm_ps", bufs=2, space="PSUM"))
            rowp = fctx.enter_context(tc.tile_pool(name="rows", bufs=2))

            for side, sT, FD in (("x", xT, n_rows), ("z", zT, m)):
                for k in range(K):
                    dst = TX[k] if side == "x" else BZ[k]
                    if side == "x":
                        msetr(dst[H : AUG, :], 1.0)
                    else:
                        msetr(dst[H : H + 2, :], 1.0)
                        vrow = rowp.tile([1, CH], MDT, tag="vrow", name="vrow")
                        msetr(vrow, float(-lws[k] / cks[k]))
                    for c0 in range(0, FD, CH):
                        cw = min(CH, FD - c0)

                        def mm(ps_, lhsT, rhs):
                            for j in range(0, cw, MMF):
                                jw = min(MMF, cw - j)
                                nc.tensor.matmul(ps_[:, j : j + jw], lhsT,
                                                 rhs[:, j : j + jw],
                                                 start=True, stop=True)

                        tb = TBUF[rot[0] % NTH]
                        hb = HBUF[rot[0] % NTH]
                        rot[0] += 1
                        # L1: u = W1 @ xT ; t = e^u ; h = ln(e^{b1} t + 1)
                        u1 = mps.tile([H, CH], F32, tag="u")
                        for j in range(0, cw, MMF):
                            jw = min(MMF, cw - j)
                            nc.tensor.matmul(u1[:, j : j + jw], SW1[k],
                                             sT[:, c0 + j : c0 + j + jw],
                                             start=True, stop=True)
                        nc.scalar.activation(tb[0:H, :cw], u1[:, :cw], AF.Exp)
                        p1 = mps.tile([H, CH], F32, tag="u")
                        mm(p1, SD1[k], tb)
                        nc.scalar.activation(hb[0:H, :cw], p1[:, :cw], AF.Ln)

                        # L2: u = W2 @ h1 + b2 ; softplus
                        u2 = mps.tile([H, CH], F32, tag="u")
                        mm(u2, SWB2[k], hb)
                        nc.scalar.activation(tb[0:H, :cw], u2[:, :cw], AF.Exp)
                        p2 = mps.tile([H, CH], F32, tag="u")
                        mm(p2, SI, tb)
                        nc.scalar.activation(hb[0:H, :cw], p2[:, :cw], AF.Ln)

                        # L3: f = W3 @ h2 + b3
                        u3 = mps.tile([H, CH], F32, tag="u")
                        mm(u3, SWB3[k], hb)
                        if side == "x":
                            nc.vector.tensor_copy(dst[0:H, c0 : c0 + cw], u3[:, :cw])
                        else:
                            nc.vector.tensor_scalar(
                                dst[0:H, c0 : c0 + cw], u3[:, :cw], -2.0, None, OP.mult
                            )
                        # ||f||^2 (z rows hold -2fz -> 4x, rescaled below)
                        sq = hp.tile([H, CH], MDT, tag="sqf")
                        nc.vector.tensor_mul(
                            sq[:, :cw], dst[0:H, c0 : c0 + cw], dst[0:H, c0 : c0 + cw]
                        )
                        nrow = rowp.tile([1, CH], F32, tag="nrow", name="nrow")
                        for j in range(0, cw, MMF):
                            jw = min(MMF, cw - j)
                            np_ps = nps.tile([1, MMF], F32, tag="n_ps")
                            nc.tensor.matmul(np_ps[:, :jw], ones_col, sq[:, j : j + jw],
                                             start=True, stop=True)
                            nc.vector.tensor_copy(nrow[:, j : j + jw], np_ps[:, :jw])
                        # f32r split of the norm row: n = s0 + s1
                        if side == "z":
                            nc.vector.tensor_scalar(nrow[:, :cw], nrow[:, :cw], 0.25, None, OP.mult)
                        s0 = rowp.tile([1, CH], MDT, tag="s0", name="s0")
                        s1 = rowp.tile([1, CH], MDT, tag="s1", name="s1")
                        nc.vector.tensor_copy(s0[:, :cw], nrow[:, :cw])
                        nc.vector.tensor_tensor(s1[:, :cw], nrow[:, :cw], s0[:, :cw], OP.subtract)
                        if side == "x":
                            nc.sync.dma_start(out=dst[H : H + 1, c0 : c0 + cw], in_=s0[:, :cw])
                            nc.sync.dma_start(out=dst[H + 1 : H + 2, c0 : c0 + cw], in_=s1[:, :cw])
                        else:
                            nc.sync.dma_start(out=dst[H + 2 : H + 3, c0 : c0 + cw], in_=s0[:, :cw])
                            nc.sync.dma_start(out=dst[H + 3 : H + 4, c0 : c0 + cw], in_=s1[:, :cw])
                            nc.sync.dma_start(out=dst[H + 4 : H + 5, c0 : c0 + cw], in_=vrow[:, :cw])

        # ---------------- Phase G: Gram + exp + k-sum ----------------
        tfctx.close()

        EDT = BF16 if E_BF16 else F32
        with ExitStack() as gctx:
            gps = gctx.enter_context(tc.tile_pool(name="gram_ps", bufs=2, space="PSUM"))
            ep = gctx.enter_context(tc.tile_pool(name="epool", bufs=2))  # e0/e1/e2/t01 tags x2
            op_ = gctx.enter_context(tc.tile_pool(name="opool", bufs=3))

            for i in range(n_rows // P):
                n0 = i * P
                for h0 in range(0, m, hm):
                    es = []
                    for k in range(K):
                        ps = gps.tile([P, hm], F32, tag="gram")
                        for mt in range(0, hm, MMF):
                            nc.tensor.matmul(
                                ps[:, mt : mt + MMF],
                                TX[k][:, n0 : n0 + P],
                                BZ[k][:, h0 + mt : h0 + mt + MMF],
                                start=True, stop=True,
                            )
                        e = ep.tile([P, hm], EDT, tag=f"e{k}", name=f"e{k}")
                        nc.scalar.activation(e, ps, AF.Exp, scale=float(-cks[k]))
                        es.append(e)
                    t01 = ep.tile([P, hm], EDT, tag="t01")
                    nc.vector.tensor_tensor(t01, es[0], es[1], OP.add)
                    ot = op_.tile([P, hm], F32, tag="ot")
                    nc.vector.tensor_tensor(ot, t01, es[2], OP.add)
                    nc.sync.dma_start(out=out[n0 : n0 + P, h0 : h0 + hm], in_=ot)

    _split_overfull_waits(nc)
    nc.finalize()
    return nc


def _host_prep(inputs):
    ls = np.asarray(inputs["log_sigma"], np.float64)
    kw = np.asarray(inputs["kernel_weights"], np.float64)
    cks = 1.0 / (2.0 * np.power(10.0, ls))
    w = np.exp(kw - kw.max())
    w = w / w.sum()
    lws = np.log(w)
    return cks, lws


def run(inputs, trace=False, n_cores=N_CORES):
    cks, lws = _host_prep(inputs)
    nc = build_program(NROWS, M, cks, lws)
    x = np.ascontiguousarray(np.asarray(inputs["x"], np.float32))
    shared = {
        name: np.ascontiguousarray(np.asarray(inputs[name], np.float32))
        for name in ("z", "W1", "b1", "W2", "b2", "W3", "b3")
    }
    in_maps = [
        {"x": x[c * NROWS : (c + 1) * NROWS], **shared} for c in range(n_cores)
    ]
    res = run_bass_kernel_spmd(nc, in_maps, list(range(n_cores)), trace=trace)
    outs = [res.results[c]["out"] for c in range(n_cores)]
    return np.concatenate(outs, axis=0), res


def kernel(**inputs) -> np.ndarray:
    out, _ = run(inputs, trace=False)
    return out

